# revision 9
# baseline (speedup 1.0000x reference)
"""Trainium2 Bass kernel for nn_BasicBlock_81166291960009.

Spatially-gated residual BasicBlock (topk_masking):
  logit_i = conv(x, mask_i_w) + mask_i_b        (64->1ch, 3x3)
  m_i = sigmoid(logit_i) * (logit_i >= 0)
  nm_i = gauss3x3(m_i, sigma_i)
  out1 = relu(bn1(conv1(x))) * nm1
  out  = relu(bn2(conv2(out1)) * nm2 + x)

Sharding: data-parallel, one sample per NeuronCore (N=8 over 8 cores),
weights replicated. No cross-core communication.

Device mapping (per core, C=64, H=W=128):
 - x stored padded 130x130 (zero border), channels on partitions; a
   row-shifted duplicate on partitions 64..127 lets 3x3 taps pair up in
   K=128 matmuls (6 matmuls per 512-pos chunk instead of 9).
 - convs in bf16 with fp32 PSUM accumulation, column-tiled
   tile_position (0,0)/(0,64) so even/odd chunks run concurrently.
 - mask logits computed EXACTLY: 4-term bf16 split (x_hi/x_lo,
   mw_hi/mw_lo) accumulated in fp32 PSUM (M=18 = 2 masks x 9 taps,
   unshifted rhs), evicted fp32, then 18 shift-folded reshape DMAs to
   [128h x 128w] maps and an fp32 DVE tree-sum. Thresholding uses the
   logit sign, so hard-mask decisions match the fp32 reference.
 - gaussian: separable; vertical = split-bf16 tridiagonal matmul,
   horizontal = fp32 DVE shifted adds; per-channel broadcast of
   nm (hi+lo bf16 rows) via one K=4 selector matmul per chunk.
 - BN folded into conv weights/bias host-side. Host prep is tiny
   tensors / cheap dtype casts only.
"""

import os
import sys

for _p in ("/opt/trn_rl_repo", "/root/.axon_site/_ro/trn_rl_repo"):
    if os.path.isdir(_p) and _p not in sys.path:
        sys.path.append(_p)

import numpy as np
import ml_dtypes

import concourse.bass as bass
import concourse.bacc as bacc
import concourse.tile as tile
import concourse.mybir as mybir
from concourse import bass_utils

dt = mybir.dt
AF = mybir.ActivationFunctionType
ALU = mybir.AluOpType
BF16 = ml_dtypes.bfloat16

C = 64            # channels
H = W = 128       # spatial
P = 130           # padded pitch
NPOS = H * W      # 16384
NPAD = P * P      # 16900
CHUNK = 512       # positions per PSUM bank (4 image rows)
NCHUNK = NPOS // CHUNK   # 32
EPS_BN = 1e-5
N_CORES = 8

_COMPILED = None


def _bf16_split(a):
    hi = a.astype(BF16)
    lo = (a.astype(np.float32) - hi.astype(np.float32)).astype(BF16)
    return hi, lo


def _build():
    """Build + compile the single-core Bass module (shapes fixed)."""
    nc = bacc.Bacc("TRN2", target_bir_lowering=False, debug=False,
                   num_devices=N_CORES)

    f32, bf = dt.float32, dt.bfloat16

    def din(name, shape, dty=bf):
        return nc.dram_tensor(name, shape, dty, kind="ExternalInput")

    # per-core sample data
    xhi_d = din("xhi", [C, H, W])                    # bf16
    xlo_d = din("xlo", [C, H, W])                    # bf16
    xres_d = din("xres", [C, H, W], f32)             # exact residual
    # conv weights: per dx, stacked pair (ky0,ky1) [128,64] + single ky2
    w1p_d = din("w1p", [3, 2 * C, C])
    w1s_d = din("w1s", [3, C, C])
    w2p_d = din("w2p", [3, 2 * C, C])
    w2s_d = din("w2s", [3, C, C])
    # mask weights [64, 18] hi/lo (cols: mask1 taps 0-8, mask2 taps 9-17)
    mwhi_d = din("mwhi", [C, 18])
    mwlo_d = din("mwlo", [C, 18])
    # gaussian vertical matrices (g0-prescaled tridiag), hi/lo [2,128,128]
    syhi_d = din("syhi", [2, H, H])
    sylo_d = din("sylo", [2, H, H])
    sel_d = din("sel", [4, 2 * C])                   # bcast selector lhsT
    # per-partition scalars
    b1dup_d = din("b1dup", [2 * C], f32)
    b2dup_d = din("b2dup", [2 * C], f32)
    mb_d = din("mb", [2, H], f32)                    # mask biases replicated
    grat_d = din("grat", [2, H], f32)                # g1/g0 per mask
    out_d = nc.dram_tensor("out", [C, H, W], f32, kind="ExternalOutput")

    with tile.TileContext(nc) as tc:
        _emit(nc, tc, locals())
    nc.compile()
    return nc


def _emit(nc, tc, d):
    f32, bf = dt.float32, dt.bfloat16
    from contextlib import ExitStack
    ctx = ExitStack()

    big = ctx.enter_context(tc.tile_pool(name="big", bufs=1))
    wts = ctx.enter_context(tc.tile_pool(name="wts", bufs=1))
    sml = ctx.enter_context(tc.tile_pool(name="sml", bufs=1))
    ring = ctx.enter_context(tc.tile_pool(name="ring", bufs=2))
    stg = ctx.enter_context(tc.tile_pool(name="stg", bufs=2))
    ops = ctx.enter_context(tc.tile_pool(name="ops", bufs=1))
    psA = ctx.enter_context(tc.tile_pool(name="psA", bufs=2, space="PSUM"))
    psB = ctx.enter_context(tc.tile_pool(name="psB", bufs=2, space="PSUM"))
    psS = ctx.enter_context(tc.tile_pool(name="psS", bufs=2, space="PSUM"))

    # ---- persistent tiles ----
    x2 = big.tile([2 * C, NPAD], bf, tag="x2")        # padded x + shifted dup
    o2 = big.tile([2 * C, NPAD], bf, tag="o2")        # padded out1 + dup
    scr = big.tile([C, NPAD], f32, tag="scr")         # s-strip then xres
    maps = big.tile([H, 2 * 9 * H], f32, tag="maps")  # 18 tap maps
    nm4 = big.tile([4, NPOS], bf, tag="nm4")          # nm1hi,nm1lo,nm2hi,nm2lo
    xres = scr[0:C, 0:NPOS]

    # weights / consts
    w1p = wts.tile([2 * C, 3 * C], bf, tag="w1p")
    w1s = wts.tile([C, 3 * C], bf, tag="w1s")
    w2p = wts.tile([2 * C, 3 * C], bf, tag="w2p")
    w2s = wts.tile([C, 3 * C], bf, tag="w2s")
    mwhi = wts.tile([C, 18], bf, tag="mwhi")
    mwlo = wts.tile([C, 18], bf, tag="mwlo")
    syhi = wts.tile([H, 2 * H], bf, tag="syhi")
    sylo = wts.tile([H, 2 * H], bf, tag="sylo")
    sel = wts.tile([4, 2 * C], bf, tag="sel")
    b1dup = sml.tile([2 * C, 1], f32, tag="b1")
    b2dup = sml.tile([2 * C, 1], f32, tag="b2")
    mb = sml.tile([H, 2], f32, tag="mb")
    grat = sml.tile([H, 2], f32, tag="grat")

    def ld(t, ap):
        nc.sync.dma_start(t, ap)

    ld(w1p[:].rearrange("p (a c) -> p a c", a=3),
       d["w1p_d"].ap().rearrange("a b c -> b a c"))
    ld(w1s[:].rearrange("p (a c) -> p a c", a=3),
       d["w1s_d"].ap().rearrange("a b c -> b a c"))
    ld(w2p[:].rearrange("p (a c) -> p a c", a=3),
       d["w2p_d"].ap().rearrange("a b c -> b a c"))
    ld(w2s[:].rearrange("p (a c) -> p a c", a=3),
       d["w2s_d"].ap().rearrange("a b c -> b a c"))
    ld(mwhi[:], d["mwhi_d"].ap())
    ld(mwlo[:], d["mwlo_d"].ap())
    ld(syhi[:].rearrange("p (a c) -> p a c", a=2),
       d["syhi_d"].ap().rearrange("a b c -> b a c"))
    ld(sylo[:].rearrange("p (a c) -> p a c", a=2),
       d["sylo_d"].ap().rearrange("a b c -> b a c"))
    ld(sel[:], d["sel_d"].ap())
    ld(b1dup[:], d["b1dup_d"].ap().unsqueeze(1))
    ld(b2dup[:], d["b2dup_d"].ap().unsqueeze(1))
    ld(mb[:], d["mb_d"].ap().rearrange("a b -> b a"))
    ld(grat[:], d["grat_d"].ap().rearrange("a b -> b a"))

    x2v = x2.rearrange("p (r c) -> p r c", c=P)
    o2v = o2.rearrange("p (r c) -> p r c", c=P)
    sv = scr.rearrange("p (r c) -> p r c", c=P)
    nm4v = nm4.rearrange("p (h w) -> p h w", w=W)

    # ---- pad memsets (zero borders) ----
    for tv in (x2v, o2v):
        nc.vector.memset(tv[:, 0, :], 0)
        nc.vector.memset(tv[:, P - 1, :], 0)
        nc.vector.memset(tv[:, 1:P - 1, 0:1], 0)
        nc.vector.memset(tv[:, 1:P - 1, P - 1:P], 0)
        nc.vector.memset(tv[C:2 * C, P - 2, :], 0)
    nc.vector.memset(sv[0:18, 0, :], 0)
    nc.vector.memset(sv[0:18, P - 1, :], 0)
    nc.vector.memset(sv[0:18, 1:P - 1, 0:1], 0)
    nc.vector.memset(sv[0:18, 1:P - 1, P - 1:P], 0)

    # ---- load x: lower rows 1..128 <- x rows 0..127; upper 1..127 <- 1..127
    xh = d["xhi_d"].ap()
    nc.sync.dma_start(x2v[0:C, 1:129, 1:129], xh)
    nc.sync.dma_start(x2v[C:2 * C, 0:128, 1:129], xh)

    # =====================================================================
    # Phase 1: mask tap channels s_t (exact, 4-term bf16 split), M=18
    # =====================================================================
    xlof = d["xlo_d"].ap().rearrange("c h w -> c (h w)")
    for k in range(NCHUNK):
        par = k % 2
        tp = (0, 0) if par == 0 else (0, 64)
        r0 = 4 * k + 1
        xhi_c = x2v[0:C, r0:r0 + 4, 1:129]
        xlo_c = ring.tile([C, CHUNK], bf, tag="xlo")
        nc.sync.dma_start(xlo_c[:], xlof[:, k * CHUNK:(k + 1) * CHUNK])
        ps = psS.tile([128, CHUNK], f32, tag="s")
        po = ps[0:18, :] if par == 0 else ps[64:82, :]
        nc.tensor.matmul(po, mwhi[:], xhi_c, start=True, stop=False,
                         tile_position=tp)
        nc.tensor.matmul(po, mwlo[:], xhi_c, start=False, stop=False,
                         tile_position=tp)
        nc.tensor.matmul(po, mwhi[:], xlo_c[:], start=False, stop=False,
                         tile_position=tp)
        nc.tensor.matmul(po, mwlo[:], xlo_c[:], start=False, stop=True,
                         tile_position=tp)
        nc.scalar.copy(sv[0:18, r0:r0 + 4, 1:129], po)

    # =====================================================================
    # Phase 2: mask pipeline -> nm4 rows
    # =====================================================================
    mapsv = maps.rearrange("p (t c) -> p t c", c=H)
    for t in range(18):
        dy, dx = (t % 9) // 3 - 1, (t % 9) % 3 - 1
        src = sv[t:t + 1, 1 + dy:129 + dy, 1 + dx:129 + dx]
        nc.sync.dma_start(mapsv[:, t, :], src)

    for mi in range(2):
        mbase = 9 * mi
        u1 = ops.tile([H, 4 * H], f32, tag="u1")
        nc.vector.tensor_add(u1[:], mapsv[:, mbase:mbase + 4, :],
                             mapsv[:, mbase + 4:mbase + 8, :])
        u2 = ops.tile([H, 2 * H], f32, tag="u2")
        nc.vector.tensor_add(u2[:], u1[:, 0:2 * H], u1[:, 2 * H:4 * H])
        u3 = ops.tile([H, H], f32, tag="u3")
        nc.vector.tensor_add(u3[:], u2[:, 0:H], u2[:, H:2 * H])
        logit = ops.tile([H, H], f32, tag="logit")
        nc.vector.scalar_tensor_tensor(
            logit[:], u3[:], mb[:, mi:mi + 1], mapsv[:, mbase + 8, :],
            op0=ALU.add, op1=ALU.add)
        p = ops.tile([H, H], f32, tag="p")
        nc.scalar.activation(p[:], logit[:], AF.Sigmoid)
        m = ops.tile([H, H], f32, tag="m")
        nc.vector.scalar_tensor_tensor(m[:], logit[:], 0.0, p[:],
                                       op0=ALU.is_ge, op1=ALU.mult)
        mhi = ops.tile([H, H], bf, tag="mhi")
        nc.vector.tensor_copy(mhi[:], m[:])
        mlo = ops.tile([H, H], bf, tag="mlo")
        nc.vector.scalar_tensor_tensor(mlo[:], mhi[:], -1.0, m[:],
                                       op0=ALU.mult, op1=ALU.add)
        # vertical gaussian (g0-prescaled) via 2 col tiles x 3 split terms
        pnv = psS.tile([H, H], f32, tag="nmv")
        for cti in range(2):
            c0, c1 = (0, C) if cti == 0 else (C, 2 * C)
            tp = (0, 0) if cti == 0 else (0, 64)
            po = pnv[c0:c1, :]
            lhi = syhi[:, mi * H + c0:mi * H + c1]
            llo = sylo[:, mi * H + c0:mi * H + c1]
            nc.tensor.matmul(po, lhi, mhi[:], start=True, stop=False,
                             tile_position=tp)
            nc.tensor.matmul(po, lhi, mlo[:], start=False, stop=False,
                             tile_position=tp)
            nc.tensor.matmul(po, llo, mhi[:], start=False, stop=True,
                             tile_position=tp)
        nmv = ops.tile([H, P], f32, tag="nmv_sb")
        nc.vector.memset(nmv[:, 0:1], 0)
        nc.vector.memset(nmv[:, P - 1:P], 0)
        nc.scalar.copy(nmv[:, 1:129], pnv[:])
        t2 = ops.tile([H, H], f32, tag="t2")
        nc.vector.tensor_add(t2[:], nmv[:, 0:128], nmv[:, 2:130])
        nm = ops.tile([H, H], f32, tag="nm")
        nc.vector.scalar_tensor_tensor(nm[:], t2[:], grat[:, mi:mi + 1],
                                       nmv[:, 1:129], op0=ALU.mult,
                                       op1=ALU.add)
        nmhi = ops.tile([H, H], bf, tag="nmhi")
        nc.vector.tensor_copy(nmhi[:], nm[:])
        nmlo = ops.tile([H, H], bf, tag="nmlo")
        nc.vector.scalar_tensor_tensor(nmlo[:], nmhi[:], -1.0, nm[:],
                                       op0=ALU.mult, op1=ALU.add)
        nc.sync.dma_start(nm4v[2 * mi:2 * mi + 1, :, :], nmhi[:])
        nc.sync.dma_start(nm4v[2 * mi + 1:2 * mi + 2, :, :], nmlo[:])

    # =====================================================================
    # conv helpers
    # =====================================================================
    def conv_chunk(src2v, wp, ws, ps, par, k):
        tp = (0, 0) if par == 0 else (0, 64)
        po = ps[0:C, :] if par == 0 else ps[C:2 * C, :]
        r0 = 4 * k + 1
        for kx in range(3):
            dx = kx - 1
            rhs = src2v[:, r0 - 1:r0 + 3, 1 + dx:129 + dx]
            nc.tensor.matmul(po, wp[:, kx * C:(kx + 1) * C], rhs,
                             start=(kx == 0), stop=False, tile_position=tp)
            rhs1 = src2v[0:C, r0 + 1:r0 + 5, 1 + dx:129 + dx]
            nc.tensor.matmul(po, ws[:, kx * C:(kx + 1) * C], rhs1,
                             start=False, stop=(kx == 2), tile_position=tp)

    def bcast_chunk(mi, ps, par, k):
        tp = (0, 0) if par == 0 else (0, 64)
        po = ps[0:C, :] if par == 0 else ps[C:2 * C, :]
        rhs = nm4v[:, 4 * k:4 * k + 4, :]
        nc.tensor.matmul(po, sel[:, mi * C:(mi + 1) * C], rhs,
                         start=True, stop=True, tile_position=tp)

    # =====================================================================
    # Phase 3: conv1 + bn1 + relu + *nm1 -> o2 (bf16, padded)
    # =====================================================================
    for kp in range(NCHUNK // 2):
        pA = psA.tile([2 * C, CHUNK], f32, tag="cv")
        pB = psB.tile([2 * C, CHUNK], f32, tag="nm")
        for par in range(2):
            k = 2 * kp + par
            conv_chunk(x2v, w1p, w1s, pA, par, k)
            bcast_chunk(0, pB, par, k)
        rS = stg.tile([2 * C, CHUNK], f32, tag="ev1")
        nc.scalar.activation(rS[:], pA[:], AF.Relu, bias=b1dup[:, 0:1])
        for par in range(2):
            k = 2 * kp + par
            r0 = 4 * k + 1
            h0, h1 = (0, C) if par == 0 else (C, 2 * C)
            nc.vector.tensor_mul(
                o2v[0:C, r0:r0 + 4, 1:129],
                rS[h0:h1, :].rearrange("p (r c) -> p r c", c=W),
                pB[h0:h1, :].rearrange("p (r c) -> p r c", c=W))
        if kp % 4 == 3:
            # upper slot r <- out1 row r (= lower slot r+1); rows of this
            # quarter are all evicted by chunk 8q+7 (this kp)
            q = kp // 4
            nc.sync.dma_start(o2v[C:2 * C, 32 * q:32 * q + 32, 1:129],
                              o2v[0:C, 32 * q + 1:32 * q + 33, 1:129])

    # load xres (reuses scr; Tile orders it after the maps DMAs read s)
    nc.sync.dma_start(xres, d["xres_d"].ap().rearrange("c h w -> c (h w)"))

    # =====================================================================
    # Phase 4: conv2 + bn2 + *nm2 + residual + relu -> out
    # =====================================================================
    outf = d["out_d"].ap().rearrange("c h w -> c (h w)")
    for kp in range(NCHUNK // 2):
        pA = psA.tile([2 * C, CHUNK], f32, tag="cv")
        pB = psB.tile([2 * C, CHUNK], f32, tag="nm")
        for par in range(2):
            k = 2 * kp + par
            conv_chunk(o2v, w2p, w2s, pA, par, k)
            bcast_chunk(1, pB, par, k)
        tS = stg.tile([2 * C, CHUNK], f32, tag="ev2")
        nc.scalar.activation(tS[:], pA[:], AF.Identity, bias=b2dup[:, 0:1])
        for par in range(2):
            k = 2 * kp + par
            h0, h1 = (0, C) if par == 0 else (C, 2 * C)
            uc = stg.tile([C, CHUNK], f32, tag="uc")
            nc.vector.tensor_mul(uc[:], tS[h0:h1, :], pB[h0:h1, :])
            vc = stg.tile([C, CHUNK], f32, tag="vc")
            nc.vector.tensor_add(vc[:], uc[:],
                                 xres[:, k * CHUNK:(k + 1) * CHUNK])
            oc = stg.tile([C, CHUNK], f32, tag="oc")
            nc.scalar.activation(oc[:], vc[:], AF.Relu)
            nc.sync.dma_start(outf[:, k * CHUNK:(k + 1) * CHUNK], oc[:])

    ctx.close()


def _host_prep(inputs):
    """Fold BN, split dtypes, build constant matrices."""
    f32 = np.float32
    x = np.asarray(inputs["x"], f32)
    N = x.shape[0]

    def fold(w, gamma, beta, mean, var):
        scale = (np.asarray(gamma, f32)
                 / np.sqrt(np.asarray(var, f32) + f32(EPS_BN)))
        wf = np.asarray(w, f32) * scale[:, None, None, None]
        b = np.asarray(beta, f32) - np.asarray(mean, f32) * scale
        return wf, b

    w1f, b1 = fold(inputs["conv1_w"], inputs["bn1_gamma"], inputs["bn1_beta"],
                   inputs["bn1_mean"], inputs["bn1_var"])
    w2f, b2 = fold(inputs["conv2_w"], inputs["bn2_gamma"], inputs["bn2_beta"],
                   inputs["bn2_mean"], inputs["bn2_var"])

    def pack_conv(wf):
        wp = np.zeros((3, 2 * C, C), f32)
        wsg = np.zeros((3, C, C), f32)
        for kx in range(3):
            wp[kx, 0:C] = wf[:, :, 0, kx].T      # ky=0 (dy=-1), lower half
            wp[kx, C:2 * C] = wf[:, :, 1, kx].T  # ky=1 (dy=0) via shifted dup
            wsg[kx] = wf[:, :, 2, kx].T          # ky=2 (dy=+1)
        return wp.astype(BF16), wsg.astype(BF16)

    w1p, w1s = pack_conv(w1f)
    w2p, w2s = pack_conv(w2f)

    mw = np.zeros((C, 18), f32)
    for t in range(9):
        ky, kx = t // 3, t % 3
        mw[:, t] = np.asarray(inputs["mask1_w"], f32)[0, :, ky, kx]
        mw[:, 9 + t] = np.asarray(inputs["mask2_w"], f32)[0, :, ky, kx]
    mwhi, mwlo = _bf16_split(mw)

    syhi = np.zeros((2, H, H), BF16)
    sylo = np.zeros((2, H, H), BF16)
    grat = np.zeros((2, H), f32)
    for mi, sig in enumerate((inputs["sigma1"], inputs["sigma2"])):
        s = f32(np.asarray(sig, f32).reshape(-1)[0])
        dd = np.arange(3, dtype=f32) - f32(1.0)
        e = np.exp(-(dd * dd) / (2 * s * s)).astype(f32)
        g1d = (e / e.sum()).astype(f32)
        g0, g1 = g1d[1], g1d[0]
        sy = np.zeros((H, H), f32)
        for h in range(H):
            for dy in (-1, 0, 1):
                hh = h + dy
                if 0 <= hh < H:
                    sy[hh, h] = g0 * g1d[dy + 1]   # lhsT[h_in, h_out]
        hi, lo = _bf16_split(sy)
        syhi[mi], sylo[mi] = hi, lo
        grat[mi, :] = g1 / g0
    mb = np.zeros((2, H), f32)
    mb[0, :] = np.asarray(inputs["mask1_b"], f32).reshape(-1)[0]
    mb[1, :] = np.asarray(inputs["mask2_b"], f32).reshape(-1)[0]

    sel = np.zeros((4, 2 * C), BF16)
    sel[0, 0:C] = 1
    sel[1, 0:C] = 1
    sel[2, C:2 * C] = 1
    sel[3, C:2 * C] = 1

    shared = dict(
        w1p=w1p, w1s=w1s, w2p=w2p, w2s=w2s,
        mwhi=mwhi, mwlo=mwlo, syhi=syhi, sylo=sylo, sel=sel,
        b1dup=np.concatenate([b1, b1]).astype(f32),
        b2dup=np.concatenate([b2, b2]).astype(f32),
        mb=mb, grat=grat,
    )
    per_core = []
    for i in range(N):
        xi = x[i]
        xhi = xi.astype(BF16)
        xlo = (xi - xhi.astype(f32)).astype(BF16)
        per_core.append(dict(shared, xhi=xhi, xlo=xlo, xres=xi))
    return per_core


def kernel(**inputs):
    global _COMPILED
    if _COMPILED is None:
        _COMPILED = _build()
    nc = _COMPILED
    per_core = _host_prep(inputs)
    res = bass_utils.run_bass_kernel_spmd(
        nc, per_core, core_ids=list(range(len(per_core))))
    out = np.stack([res.results[i]["out"] for i in range(len(per_core))])
    return out.astype(np.float32)


if __name__ == "__main__":
    sys.path.insert(0, os.path.dirname(os.path.abspath(__file__)))
    import jax
    import reference
    with jax.default_device(jax.devices("cpu")[0]):
        ins = {k: np.asarray(v) for k, v in reference.setup_inputs().items()}
        exp = np.asarray(reference.reference(**ins))
    act = kernel(**ins)
    err = np.abs(act - exp)
    denom = np.abs(exp).max()
    print(f"max abs err: {err.max():.3e}  (ref scale {denom:.3f})")
    print(f"Relative error: {err.max() / denom:.3e}")


# revision 12
# speedup vs baseline: 1.6054x; 1.6054x over previous
"""Trainium2 Bass kernel for nn_BasicBlock_81166291960009.

Spatially-gated residual BasicBlock (topk_masking):
  logit_i = conv(x, mask_i_w) + mask_i_b        (64->1ch, 3x3)
  m_i = sigmoid(logit_i) * (logit_i >= 0)
  nm_i = gauss3x3(m_i, sigma_i)
  out1 = relu(bn1(conv1(x))) * nm1
  out  = relu(bn2(conv2(out1)) * nm2 + x)

Sharding: data-parallel, one sample per NeuronCore (N=8 over 8 cores),
weights replicated. No cross-core communication.

Device mapping (per core, C=64, H=W=128):
 - x stored padded 130x130 (zero border), channels on partitions; a
   row-shifted duplicate on partitions 64..127 lets 3x3 taps pair up in
   K=128 matmuls (6 matmuls per 512-pos chunk instead of 9).
 - convs in bf16 with fp32 PSUM accumulation, column-tiled
   tile_position (0,0)/(0,64) so even/odd chunks run concurrently.
 - mask logits computed EXACTLY: 4-term bf16 split (x_hi/x_lo,
   mw_hi/mw_lo) accumulated in fp32 PSUM (M=18 = 2 masks x 9 taps,
   unshifted rhs), evicted fp32, then 18 shift-folded reshape DMAs to
   [128h x 128w] maps and an fp32 DVE tree-sum. Thresholding uses the
   logit sign, so hard-mask decisions match the fp32 reference.
 - gaussian: separable; vertical = split-bf16 tridiagonal matmul,
   horizontal = fp32 DVE shifted adds; per-channel broadcast of
   nm (hi+lo bf16 rows) via one K=4 selector matmul per chunk.
 - BN folded into conv weights/bias host-side. Host prep is tiny
   tensors / cheap dtype casts only.
"""

import os
import sys

for _p in ("/opt/trn_rl_repo", "/root/.axon_site/_ro/trn_rl_repo"):
    if os.path.isdir(_p) and _p not in sys.path:
        sys.path.append(_p)

import numpy as np
import ml_dtypes

import concourse.bass as bass
import concourse.bacc as bacc
import concourse.tile as tile
import concourse.mybir as mybir
from concourse import bass_utils

dt = mybir.dt
AF = mybir.ActivationFunctionType
ALU = mybir.AluOpType
BF16 = ml_dtypes.bfloat16

C = 64            # channels
H = W = 128       # spatial
P = 130           # padded pitch
NPOS = H * W      # 16384
NPAD = P * P      # 16900
CHUNK = 512       # positions per PSUM bank (4 image rows)
NCHUNK = NPOS // CHUNK   # 32
EPS_BN = 1e-5
N_CORES = 8

_COMPILED = None


def _bf16_split(a):
    hi = a.astype(BF16)
    lo = (a.astype(np.float32) - hi.astype(np.float32)).astype(BF16)
    return hi, lo


def _build():
    """Build + compile the single-core Bass module (shapes fixed)."""
    nc = bacc.Bacc("TRN2", target_bir_lowering=False, debug=False,
                   num_devices=N_CORES)

    f32, bf = dt.float32, dt.bfloat16

    def din(name, shape, dty=bf):
        return nc.dram_tensor(name, shape, dty, kind="ExternalInput")

    # per-core sample data: xstack = [bf16(x); bf16(x - bf16(x))]
    xstack_d = din("xstack", [2 * C, H, W])
    xres_d = din("xres", [C, H, W], f32)             # exact residual
    # conv weights: per dx, stacked pair (ky0,ky1) [128,64] + single ky2
    # (single padded to K=128 with zero upper half: uniform-K matmul stream)
    w1p_d = din("w1p", [3, 2 * C, C])
    w1s_d = din("w1s", [3, 2 * C, C])
    w2p_d = din("w2p", [3, 2 * C, C])
    w2s_d = din("w2s", [3, 2 * C, C])
    # mask weights stacked for K=128 split matmuls:
    # mwA = [mw_hi; mw_lo], mwB = [mw_lo; mw_hi]
    mwA_d = din("mwA", [2 * C, 18])
    mwB_d = din("mwB", [2 * C, 18])
    # gaussian vertical matrices (g0-prescaled tridiag), hi/lo [2,128,128]
    syhi_d = din("syhi", [2, H, H])
    sylo_d = din("sylo", [2, H, H])
    sel_d = din("sel", [2 * C, 2 * C])               # bcast selector lhsT
    # per-partition scalars
    b1dup_d = din("b1dup", [2 * C], f32)
    b2dup_d = din("b2dup", [2 * C], f32)
    mb_d = din("mb", [2, H], f32)                    # mask biases replicated
    grat_d = din("grat", [2, H], f32)                # g1/g0 per mask
    out_d = nc.dram_tensor("out", [C, H, W], f32, kind="ExternalOutput")

    with tile.TileContext(nc) as tc:
        _emit(nc, tc, locals())
    nc.compile()
    return nc


def _emit(nc, tc, d):
    f32, bf = dt.float32, dt.bfloat16
    from contextlib import ExitStack
    ctx = ExitStack()

    big = ctx.enter_context(tc.tile_pool(name="big", bufs=1))
    wts = ctx.enter_context(tc.tile_pool(name="wts", bufs=1))
    sml = ctx.enter_context(tc.tile_pool(name="sml", bufs=1))
    ring = ctx.enter_context(tc.tile_pool(name="ring", bufs=2))
    stg = ctx.enter_context(tc.tile_pool(name="stg", bufs=2))
    ops = ctx.enter_context(tc.tile_pool(name="ops", bufs=1))
    psA = ctx.enter_context(tc.tile_pool(name="psA", bufs=2, space="PSUM"))
    psB = ctx.enter_context(tc.tile_pool(name="psB", bufs=2, space="PSUM"))
    psS = ctx.enter_context(tc.tile_pool(name="psS", bufs=2, space="PSUM"))

    # ---- persistent tiles ----
    x2 = big.tile([2 * C, NPAD], bf, tag="x2")        # padded x + shifted dup
    o2 = big.tile([2 * C, NPAD], bf, tag="o2")        # padded out1 + dup
    scr = big.tile([C, NPAD], f32, tag="scr")         # s-strip then xres
    maps = big.tile([H, 2 * 9 * H], f32, tag="maps")  # 18 tap maps
    nm4 = big.tile([2 * C, NPOS], bf, tag="nm4")      # rows0-3 nm hi/lo, rest 0
    xres = scr[0:C, 0:NPOS]

    # weights / consts
    w1p = wts.tile([2 * C, 3 * C], bf, tag="w1p")
    w1s = wts.tile([2 * C, 3 * C], bf, tag="w1s")
    w2p = wts.tile([2 * C, 3 * C], bf, tag="w2p")
    w2s = wts.tile([2 * C, 3 * C], bf, tag="w2s")
    mwA = wts.tile([2 * C, 18], bf, tag="mwA")
    mwB = wts.tile([2 * C, 18], bf, tag="mwB")
    syhi = wts.tile([H, 2 * H], bf, tag="syhi")
    sylo = wts.tile([H, 2 * H], bf, tag="sylo")
    sel = wts.tile([2 * C, 2 * C], bf, tag="sel")
    b1dup = sml.tile([2 * C, 1], f32, tag="b1")
    b2dup = sml.tile([2 * C, 1], f32, tag="b2")
    mb = sml.tile([H, 2], f32, tag="mb")
    grat = sml.tile([H, 2], f32, tag="grat")

    def ld(t, ap):
        nc.sync.dma_start(t, ap)

    ld(w1p[:].rearrange("p (a c) -> p a c", a=3),
       d["w1p_d"].ap().rearrange("a b c -> b a c"))
    ld(w1s[:].rearrange("p (a c) -> p a c", a=3),
       d["w1s_d"].ap().rearrange("a b c -> b a c"))
    ld(w2p[:].rearrange("p (a c) -> p a c", a=3),
       d["w2p_d"].ap().rearrange("a b c -> b a c"))
    ld(w2s[:].rearrange("p (a c) -> p a c", a=3),
       d["w2s_d"].ap().rearrange("a b c -> b a c"))
    ld(mwA[:], d["mwA_d"].ap())
    ld(mwB[:], d["mwB_d"].ap())
    ld(syhi[:].rearrange("p (a c) -> p a c", a=2),
       d["syhi_d"].ap().rearrange("a b c -> b a c"))
    ld(sylo[:].rearrange("p (a c) -> p a c", a=2),
       d["sylo_d"].ap().rearrange("a b c -> b a c"))
    ld(sel[:], d["sel_d"].ap())
    ld(b1dup[:], d["b1dup_d"].ap().unsqueeze(1))
    ld(b2dup[:], d["b2dup_d"].ap().unsqueeze(1))
    ld(mb[:], d["mb_d"].ap().rearrange("a b -> b a"))
    ld(grat[:], d["grat_d"].ap().rearrange("a b -> b a"))

    x2v = x2.rearrange("p (r c) -> p r c", c=P)
    o2v = o2.rearrange("p (r c) -> p r c", c=P)
    sv = scr.rearrange("p (r c) -> p r c", c=P)
    nm4v = nm4.rearrange("p (h w) -> p h w", w=W)

    # ---- pad memsets (zero borders) ----
    for tv in (x2v, o2v):
        nc.vector.memset(tv[:, 0, :], 0)
        nc.vector.memset(tv[:, P - 1, :], 0)
        nc.vector.memset(tv[:, 1:P - 1, 0:1], 0)
        nc.vector.memset(tv[:, 1:P - 1, P - 1:P], 0)
        nc.vector.memset(tv[C:2 * C, P - 2, :], 0)
    nc.vector.memset(sv[0:18, 0, :], 0)
    nc.vector.memset(sv[0:18, P - 1, :], 0)
    nc.vector.memset(sv[0:18, 1:P - 1, 0:1], 0)
    nc.vector.memset(sv[0:18, 1:P - 1, P - 1:P], 0)

    # zero the padding rows of the bcast operand (read by K=128 bcast matmul)
    nc.gpsimd.memset(nm4[:], 0)   # rows 0-3 overwritten by nm gathers

    # ---- load x: lower rows 1..128 <- x rows 0..127; upper slot r <- row r
    xh = d["xstack_d"].ap()[0:C, :, :]
    nc.sync.dma_start(x2v[0:C, 1:129, 1:129], xh)
    nc.sync.dma_start(x2v[C:2 * C, 0:128, 1:129], xh)

    # =====================================================================
    # Phase 1: mask tap channels s_t (exact, 4-term bf16 split), M=18
    # =====================================================================
    xsf = d["xstack_d"].ap().rearrange("c h w -> c (h w)")
    for kb in range(NCHUNK // 2):
        xst = ring.tile([2 * C, 2 * CHUNK], bf, tag="xst")
        nc.sync.dma_start(xst[:], xsf[:, kb * 2 * CHUNK:(kb + 1) * 2 * CHUNK])
        for j in range(2):
            k = 2 * kb + j
            r0 = 4 * k + 1
            xc = xst[:, j * CHUNK:(j + 1) * CHUNK]
            ps = psS.tile([128, CHUNK], f32, tag="s")
            po = ps[0:18, :]
            # s = mwhi.x_hi + mwlo.x_lo  (MM A)  +  mwlo.x_hi + mwhi.x_lo (MM B)
            nc.tensor.matmul(po, mwA[:], xc, start=True, stop=False,
                             tile_position=(0, 0))
            nc.tensor.matmul(po, mwB[:], xc, start=False, stop=True,
                             tile_position=(0, 0))
            nc.scalar.copy(sv[0:18, r0:r0 + 4, 1:129], po)

    # =====================================================================
    # Phase 2: mask pipeline -> nm4 rows
    # =====================================================================
    mapsv = maps.rearrange("p (t c) -> p t c", c=H)
    for t in range(18):
        dy, dx = (t % 9) // 3 - 1, (t % 9) % 3 - 1
        src = sv[t:t + 1, 1 + dy:129 + dy, 1 + dx:129 + dx]
        nc.scalar.dma_start(mapsv[:, t, :], src)

    for mi in range(2):
        mbase = 9 * mi
        u1 = ops.tile([H, 4 * H], f32, tag="u1")
        nc.vector.tensor_add(u1[:], mapsv[:, mbase:mbase + 4, :],
                             mapsv[:, mbase + 4:mbase + 8, :])
        u2 = ops.tile([H, 2 * H], f32, tag="u2")
        nc.vector.tensor_add(u2[:], u1[:, 0:2 * H], u1[:, 2 * H:4 * H])
        u3 = ops.tile([H, H], f32, tag="u3")
        nc.vector.tensor_add(u3[:], u2[:, 0:H], u2[:, H:2 * H])
        logit = ops.tile([H, H], f32, tag="logit")
        nc.vector.scalar_tensor_tensor(
            logit[:], u3[:], mb[:, mi:mi + 1], mapsv[:, mbase + 8, :],
            op0=ALU.add, op1=ALU.add)
        p = ops.tile([H, H], f32, tag="p")
        nc.scalar.activation(p[:], logit[:], AF.Sigmoid)
        m = ops.tile([H, H], f32, tag="m")
        nc.vector.scalar_tensor_tensor(m[:], logit[:], 0.0, p[:],
                                       op0=ALU.is_ge, op1=ALU.mult)
        mhi = ops.tile([H, H], bf, tag="mhi")
        nc.vector.tensor_copy(mhi[:], m[:])
        mlo = ops.tile([H, H], bf, tag="mlo")
        nc.vector.scalar_tensor_tensor(mlo[:], mhi[:], -1.0, m[:],
                                       op0=ALU.mult, op1=ALU.add)
        # vertical gaussian (g0-prescaled) via 2 col tiles x 3 split terms
        pnv = psS.tile([H, H], f32, tag="nmv")
        for cti in range(2):
            c0, c1 = (0, C) if cti == 0 else (C, 2 * C)
            tp = (0, 0) if cti == 0 else (0, 64)
            po = pnv[c0:c1, :]
            lhi = syhi[:, mi * H + c0:mi * H + c1]
            llo = sylo[:, mi * H + c0:mi * H + c1]
            nc.tensor.matmul(po, lhi, mhi[:], start=True, stop=False,
                             tile_position=tp)
            nc.tensor.matmul(po, lhi, mlo[:], start=False, stop=False,
                             tile_position=tp)
            nc.tensor.matmul(po, llo, mhi[:], start=False, stop=True,
                             tile_position=tp)
        nmv = ops.tile([H, P], f32, tag="nmv_sb")
        nc.vector.memset(nmv[:, 0:1], 0)
        nc.vector.memset(nmv[:, P - 1:P], 0)
        nc.scalar.copy(nmv[:, 1:129], pnv[:])
        t2 = ops.tile([H, H], f32, tag="t2")
        nc.vector.tensor_add(t2[:], nmv[:, 0:128], nmv[:, 2:130])
        nm = ops.tile([H, H], f32, tag="nm")
        nc.vector.scalar_tensor_tensor(nm[:], t2[:], grat[:, mi:mi + 1],
                                       nmv[:, 1:129], op0=ALU.mult,
                                       op1=ALU.add)
        nmhi = ops.tile([H, H], bf, tag="nmhi")
        nc.vector.tensor_copy(nmhi[:], nm[:])
        nmlo = ops.tile([H, H], bf, tag="nmlo")
        nc.vector.scalar_tensor_tensor(nmlo[:], nmhi[:], -1.0, nm[:],
                                       op0=ALU.mult, op1=ALU.add)
        nc.scalar.dma_start(nm4v[2 * mi:2 * mi + 1, :, :], nmhi[:])
        nc.scalar.dma_start(nm4v[2 * mi + 1:2 * mi + 2, :, :], nmlo[:])

    # =====================================================================
    # conv helpers
    # =====================================================================
    def conv_chunk(src2v, wp, ws, ps, par, k):
        tp = (0, 0) if par == 0 else (0, 64)
        po = ps[0:C, :] if par == 0 else ps[C:2 * C, :]
        r0 = 4 * k + 1
        for kx in range(3):
            dx = kx - 1
            rhs = src2v[:, r0 - 1:r0 + 3, 1 + dx:129 + dx]
            nc.tensor.matmul(po, wp[:, kx * C:(kx + 1) * C], rhs,
                             start=(kx == 0), stop=False, tile_position=tp)
            rhs1 = src2v[:, r0 + 1:r0 + 5, 1 + dx:129 + dx]
            nc.tensor.matmul(po, ws[:, kx * C:(kx + 1) * C], rhs1,
                             start=False, stop=(kx == 2), tile_position=tp)

    def bcast_chunk(mi, ps, par, k):
        tp = (0, 0) if par == 0 else (0, 64)
        po = ps[0:C, :] if par == 0 else ps[C:2 * C, :]
        rhs = nm4v[:, 4 * k:4 * k + 4, :]
        nc.tensor.matmul(po, sel[:, mi * C:(mi + 1) * C], rhs,
                         start=True, stop=True, tile_position=tp)

    # =====================================================================
    # Phase 3: conv1 + bn1 + relu + *nm1 -> o2 (bf16, padded)
    # =====================================================================
    for kp in range(NCHUNK // 2):
        pA = psA.tile([2 * C, CHUNK], f32, tag="cv")
        pB = psB.tile([2 * C, CHUNK], f32, tag="nm")
        for par in range(2):
            k = 2 * kp + par
            conv_chunk(x2v, w1p, w1s, pA, par, k)
            bcast_chunk(0, pB, par, k)
        rS = stg.tile([2 * C, CHUNK], f32, tag="ev")
        nc.scalar.activation(rS[:], pA[:], AF.Relu, bias=b1dup[:, 0:1])
        for par in range(2):
            k = 2 * kp + par
            r0 = 4 * k + 1
            h0, h1 = (0, C) if par == 0 else (C, 2 * C)
            nc.vector.tensor_mul(
                o2v[0:C, r0:r0 + 4, 1:129],
                rS[h0:h1, :].rearrange("p (r c) -> p r c", c=W),
                pB[h0:h1, :].rearrange("p (r c) -> p r c", c=W))
        if kp % 4 == 3:
            # upper slot r <- out1 row r (= lower slot r+1); rows of this
            # quarter are all evicted by chunk 8q+7 (this kp)
            q = kp // 4
            nc.sync.dma_start(o2v[C:2 * C, 32 * q:32 * q + 32, 1:129],
                              o2v[0:C, 32 * q + 1:32 * q + 33, 1:129])

    # load xres (reuses scr; Tile orders it after the maps DMAs read s)
    nc.sync.dma_start(xres, d["xres_d"].ap().rearrange("c h w -> c (h w)"))

    # =====================================================================
    # Phase 4: conv2 + bn2 + *nm2 + residual + relu -> out
    # =====================================================================
    outf = d["out_d"].ap().rearrange("c h w -> c (h w)")
    for kp in range(NCHUNK // 2):
        pA = psA.tile([2 * C, CHUNK], f32, tag="cv")
        pB = psB.tile([2 * C, CHUNK], f32, tag="nm")
        for par in range(2):
            k = 2 * kp + par
            conv_chunk(o2v, w2p, w2s, pA, par, k)
            bcast_chunk(1, pB, par, k)
        tS = stg.tile([2 * C, CHUNK], f32, tag="ev")
        nc.scalar.activation(tS[:], pA[:], AF.Identity, bias=b2dup[:, 0:1])
        for par in range(2):
            k = 2 * kp + par
            h0, h1 = (0, C) if par == 0 else (C, 2 * C)
            uc = stg.tile([C, CHUNK], f32, tag="uc")
            nc.vector.tensor_mul(uc[:], tS[h0:h1, :], pB[h0:h1, :])
            vc = stg.tile([C, CHUNK], f32, tag="vc")
            nc.vector.tensor_add(vc[:], uc[:],
                                 xres[:, k * CHUNK:(k + 1) * CHUNK])
            oc = stg.tile([C, CHUNK], f32, tag="oc")
            nc.scalar.activation(oc[:], vc[:], AF.Relu)
            nc.gpsimd.dma_start(outf[:, k * CHUNK:(k + 1) * CHUNK], oc[:])

    ctx.close()


def _host_prep(inputs):
    """Fold BN, split dtypes, build constant matrices."""
    f32 = np.float32
    x = np.asarray(inputs["x"], f32)
    N = x.shape[0]

    def fold(w, gamma, beta, mean, var):
        scale = (np.asarray(gamma, f32)
                 / np.sqrt(np.asarray(var, f32) + f32(EPS_BN)))
        wf = np.asarray(w, f32) * scale[:, None, None, None]
        b = np.asarray(beta, f32) - np.asarray(mean, f32) * scale
        return wf, b

    w1f, b1 = fold(inputs["conv1_w"], inputs["bn1_gamma"], inputs["bn1_beta"],
                   inputs["bn1_mean"], inputs["bn1_var"])
    w2f, b2 = fold(inputs["conv2_w"], inputs["bn2_gamma"], inputs["bn2_beta"],
                   inputs["bn2_mean"], inputs["bn2_var"])

    def pack_conv(wf):
        wp = np.zeros((3, 2 * C, C), f32)
        wsg = np.zeros((3, 2 * C, C), f32)
        for kx in range(3):
            wp[kx, 0:C] = wf[:, :, 0, kx].T      # ky=0 (dy=-1), lower half
            wp[kx, C:2 * C] = wf[:, :, 1, kx].T  # ky=1 (dy=0) via shifted dup
            wsg[kx, 0:C] = wf[:, :, 2, kx].T     # ky=2 (dy=+1); upper half 0
        return wp.astype(BF16), wsg.astype(BF16)

    w1p, w1s = pack_conv(w1f)
    w2p, w2s = pack_conv(w2f)

    mw = np.zeros((C, 18), f32)
    for t in range(9):
        ky, kx = t // 3, t % 3
        mw[:, t] = np.asarray(inputs["mask1_w"], f32)[0, :, ky, kx]
        mw[:, 9 + t] = np.asarray(inputs["mask2_w"], f32)[0, :, ky, kx]
    mwhi, mwlo = _bf16_split(mw)
    mwA = np.concatenate([mwhi, mwlo], axis=0)
    mwB = np.concatenate([mwlo, mwhi], axis=0)

    syhi = np.zeros((2, H, H), BF16)
    sylo = np.zeros((2, H, H), BF16)
    grat = np.zeros((2, H), f32)
    for mi, sig in enumerate((inputs["sigma1"], inputs["sigma2"])):
        s = f32(np.asarray(sig, f32).reshape(-1)[0])
        dd = np.arange(3, dtype=f32) - f32(1.0)
        e = np.exp(-(dd * dd) / (2 * s * s)).astype(f32)
        g1d = (e / e.sum()).astype(f32)
        g0, g1 = g1d[1], g1d[0]
        sy = np.zeros((H, H), f32)
        for h in range(H):
            for dy in (-1, 0, 1):
                hh = h + dy
                if 0 <= hh < H:
                    sy[hh, h] = g0 * g1d[dy + 1]   # lhsT[h_in, h_out]
        hi, lo = _bf16_split(sy)
        syhi[mi], sylo[mi] = hi, lo
        grat[mi, :] = g1 / g0
    mb = np.zeros((2, H), f32)
    mb[0, :] = np.asarray(inputs["mask1_b"], f32).reshape(-1)[0]
    mb[1, :] = np.asarray(inputs["mask2_b"], f32).reshape(-1)[0]

    sel = np.zeros((2 * C, 2 * C), BF16)
    sel[0, 0:C] = 1
    sel[1, 0:C] = 1
    sel[2, C:2 * C] = 1
    sel[3, C:2 * C] = 1

    shared = dict(
        w1p=w1p, w1s=w1s, w2p=w2p, w2s=w2s,
        mwA=mwA, mwB=mwB, syhi=syhi, sylo=sylo, sel=sel,
        b1dup=np.concatenate([b1, b1]).astype(f32),
        b2dup=np.concatenate([b2, b2]).astype(f32),
        mb=mb, grat=grat,
    )
    per_core = []
    for i in range(N):
        xi = x[i]
        xhi = xi.astype(BF16)
        xlo = (xi - xhi.astype(f32)).astype(BF16)
        xstack = np.concatenate([xhi, xlo], axis=0)
        per_core.append(dict(shared, xstack=xstack, xres=xi))
    return per_core


def kernel(**inputs):
    global _COMPILED
    if _COMPILED is None:
        _COMPILED = _build()
    nc = _COMPILED
    per_core = _host_prep(inputs)
    res = bass_utils.run_bass_kernel_spmd(
        nc, per_core, core_ids=list(range(len(per_core))))
    out = np.stack([res.results[i]["out"] for i in range(len(per_core))])
    return out.astype(np.float32)


if __name__ == "__main__":
    sys.path.insert(0, os.path.dirname(os.path.abspath(__file__)))
    import jax
    import reference
    with jax.default_device(jax.devices("cpu")[0]):
        ins = {k: np.asarray(v) for k, v in reference.setup_inputs().items()}
        exp = np.asarray(reference.reference(**ins))
    act = kernel(**ins)
    err = np.abs(act - exp)
    denom = np.abs(exp).max()
    print(f"max abs err: {err.max():.3e}  (ref scale {denom:.3f})")
    print(f"Relative error: {err.max() / denom:.3e}")


# revision 13
# speedup vs baseline: 1.7407x; 1.0843x over previous
"""Trainium2 Bass kernel for nn_BasicBlock_81166291960009.

Spatially-gated residual BasicBlock (topk_masking):
  logit_i = conv(x, mask_i_w) + mask_i_b        (64->1ch, 3x3)
  m_i = sigmoid(logit_i) * (logit_i >= 0)
  nm_i = gauss3x3(m_i, sigma_i)
  out1 = relu(bn1(conv1(x))) * nm1
  out  = relu(bn2(conv2(out1)) * nm2 + x)

Sharding: data-parallel, one sample per NeuronCore (N=8 over 8 cores),
weights replicated. No cross-core communication.

Device mapping (per core, C=64, H=W=128):
 - x stored padded 130x130 (zero border), channels on partitions; a
   row-shifted duplicate on partitions 64..127 lets 3x3 taps pair up in
   K=128 matmuls (6 matmuls per 512-pos chunk instead of 9).
 - convs in bf16 with fp32 PSUM accumulation, column-tiled
   tile_position (0,0)/(0,64) so even/odd chunks run concurrently.
 - mask logits computed EXACTLY: 4-term bf16 split (x_hi/x_lo,
   mw_hi/mw_lo) accumulated in fp32 PSUM (M=18 = 2 masks x 9 taps,
   unshifted rhs), evicted fp32, then 18 shift-folded reshape DMAs to
   [128h x 128w] maps and an fp32 DVE tree-sum. Thresholding uses the
   logit sign, so hard-mask decisions match the fp32 reference.
 - gaussian: separable; vertical = split-bf16 tridiagonal matmul,
   horizontal = fp32 DVE shifted adds; per-channel broadcast of
   nm (hi+lo bf16 rows) via one K=4 selector matmul per chunk.
 - BN folded into conv weights/bias host-side. Host prep is tiny
   tensors / cheap dtype casts only.
"""

import os
import sys

for _p in ("/opt/trn_rl_repo", "/root/.axon_site/_ro/trn_rl_repo"):
    if os.path.isdir(_p) and _p not in sys.path:
        sys.path.append(_p)

import numpy as np
import ml_dtypes

import concourse.bass as bass
import concourse.bacc as bacc
import concourse.tile as tile
import concourse.mybir as mybir
from concourse import bass_utils

dt = mybir.dt
AF = mybir.ActivationFunctionType
ALU = mybir.AluOpType
BF16 = ml_dtypes.bfloat16

C = 64            # channels
H = W = 128       # spatial
P = 130           # padded pitch
NPOS = H * W      # 16384
NPAD = P * P      # 16900
CHUNK = 512       # positions per PSUM bank (4 image rows)
NCHUNK = NPOS // CHUNK   # 32
EPS_BN = 1e-5
N_CORES = 8

_COMPILED = None


def _bf16_split(a):
    hi = a.astype(BF16)
    lo = (a.astype(np.float32) - hi.astype(np.float32)).astype(BF16)
    return hi, lo


def _build():
    """Build + compile the single-core Bass module (shapes fixed)."""
    nc = bacc.Bacc("TRN2", target_bir_lowering=False, debug=False,
                   num_devices=N_CORES)

    f32, bf = dt.float32, dt.bfloat16

    def din(name, shape, dty=bf):
        return nc.dram_tensor(name, shape, dty, kind="ExternalInput")

    # per-core sample data: xstack = [bf16(x); bf16(x - bf16(x))]
    xstack_d = din("xstack", [2 * C, H, W])
    xres_d = din("xres", [C, H, W], f32)             # exact residual
    # conv weights: per dx, stacked pair (ky0,ky1) [128,64] + single ky2
    # (single padded to K=128 with zero upper half: uniform-K matmul stream)
    w1p_d = din("w1p", [3, 2 * C, C])
    w1s_d = din("w1s", [3, 2 * C, C])
    w2p_d = din("w2p", [3, 2 * C, C])
    w2s_d = din("w2s", [3, 2 * C, C])
    # mask weights stacked for K=128 split matmuls:
    # mwA = [mw_hi; mw_lo], mwB = [mw_lo; mw_hi]
    mwA_d = din("mwA", [2 * C, 18])
    mwB_d = din("mwB", [2 * C, 18])
    # gaussian vertical matrices (g0-prescaled tridiag), hi/lo [2,128,128]
    syhi_d = din("syhi", [2, H, H])
    sylo_d = din("sylo", [2, H, H])
    sel_d = din("sel", [2 * C, 2 * C])               # bcast selector lhsT
    # per-partition scalars
    b1dup_d = din("b1dup", [2 * C], f32)
    b2dup_d = din("b2dup", [2 * C], f32)
    mb_d = din("mb", [2, H], f32)                    # mask biases replicated
    grat_d = din("grat", [2, H], f32)                # g1/g0 per mask
    out_d = nc.dram_tensor("out", [C, H, W], f32, kind="ExternalOutput")

    with tile.TileContext(nc) as tc:
        _emit(nc, tc, locals())
    nc.compile()
    return nc


def _emit(nc, tc, d):
    f32, bf = dt.float32, dt.bfloat16
    from contextlib import ExitStack
    ctx = ExitStack()

    big = ctx.enter_context(tc.tile_pool(name="big", bufs=1))
    wts = ctx.enter_context(tc.tile_pool(name="wts", bufs=1))
    sml = ctx.enter_context(tc.tile_pool(name="sml", bufs=1))
    ring = ctx.enter_context(tc.tile_pool(name="ring", bufs=2))
    stg = ctx.enter_context(tc.tile_pool(name="stg", bufs=2))
    ops = ctx.enter_context(tc.tile_pool(name="ops", bufs=1))
    psA = ctx.enter_context(tc.tile_pool(name="psA", bufs=3, space="PSUM"))
    psB = ctx.enter_context(tc.tile_pool(name="psB", bufs=3, space="PSUM"))
    psS = ctx.enter_context(tc.tile_pool(name="psS", bufs=2, space="PSUM"))

    # ---- persistent tiles ----
    x2 = big.tile([2 * C, NPAD], bf, tag="x2")        # padded x + shifted dup
    o2 = big.tile([2 * C, NPAD], bf, tag="o2")        # padded out1 + dup
    scr = big.tile([C, NPAD], f32, tag="scr")         # s-strip then xres
    maps = big.tile([H, 2 * 9 * H], f32, tag="maps")  # 18 tap maps
    nm4 = big.tile([2 * C, NPOS], bf, tag="nm4")      # rows0-3 nm hi/lo, rest 0
    xres = scr[0:C, 0:NPOS]

    # weights / consts
    w1p = wts.tile([2 * C, 3 * C], bf, tag="w1p")
    w1s = wts.tile([2 * C, 3 * C], bf, tag="w1s")
    w2p = wts.tile([2 * C, 3 * C], bf, tag="w2p")
    w2s = wts.tile([2 * C, 3 * C], bf, tag="w2s")
    mwA = wts.tile([2 * C, 18], bf, tag="mwA")
    mwB = wts.tile([2 * C, 18], bf, tag="mwB")
    syhi = wts.tile([H, 2 * H], bf, tag="syhi")
    sylo = wts.tile([H, 2 * H], bf, tag="sylo")
    sel = wts.tile([2 * C, 2 * C], bf, tag="sel")
    b1dup = sml.tile([2 * C, 1], f32, tag="b1")
    b2dup = sml.tile([2 * C, 1], f32, tag="b2")
    mb = sml.tile([H, 2], f32, tag="mb")
    grat = sml.tile([H, 2], f32, tag="grat")

    def ld(t, ap):
        nc.gpsimd.dma_start(t, ap)

    ld(w1p[:].rearrange("p (a c) -> p a c", a=3),
       d["w1p_d"].ap().rearrange("a b c -> b a c"))
    ld(w1s[:].rearrange("p (a c) -> p a c", a=3),
       d["w1s_d"].ap().rearrange("a b c -> b a c"))
    ld(w2p[:].rearrange("p (a c) -> p a c", a=3),
       d["w2p_d"].ap().rearrange("a b c -> b a c"))
    ld(w2s[:].rearrange("p (a c) -> p a c", a=3),
       d["w2s_d"].ap().rearrange("a b c -> b a c"))
    ld(mwA[:], d["mwA_d"].ap())
    ld(mwB[:], d["mwB_d"].ap())
    ld(syhi[:].rearrange("p (a c) -> p a c", a=2),
       d["syhi_d"].ap().rearrange("a b c -> b a c"))
    ld(sylo[:].rearrange("p (a c) -> p a c", a=2),
       d["sylo_d"].ap().rearrange("a b c -> b a c"))
    ld(sel[:], d["sel_d"].ap())
    ld(b1dup[:], d["b1dup_d"].ap().unsqueeze(1))
    ld(b2dup[:], d["b2dup_d"].ap().unsqueeze(1))
    ld(mb[:], d["mb_d"].ap().rearrange("a b -> b a"))
    ld(grat[:], d["grat_d"].ap().rearrange("a b -> b a"))

    x2v = x2.rearrange("p (r c) -> p r c", c=P)
    o2v = o2.rearrange("p (r c) -> p r c", c=P)
    sv = scr.rearrange("p (r c) -> p r c", c=P)
    nm4v = nm4.rearrange("p (h w) -> p h w", w=W)

    # ---- pad memsets (zero borders) ----
    for tv in (x2v, o2v):
        nc.vector.memset(tv[:, 0, :], 0)
        nc.vector.memset(tv[:, P - 1, :], 0)
        nc.vector.memset(tv[:, 1:P - 1, 0:1], 0)
        nc.vector.memset(tv[:, 1:P - 1, P - 1:P], 0)
        nc.vector.memset(tv[C:2 * C, P - 2, :], 0)
    nc.vector.memset(sv[0:18, 0, :], 0)
    nc.vector.memset(sv[0:18, P - 1, :], 0)
    nc.vector.memset(sv[0:18, 1:P - 1, 0:1], 0)
    nc.vector.memset(sv[0:18, 1:P - 1, P - 1:P], 0)

    # zero the padding rows of the bcast operand (read by K=128 bcast matmul)
    nc.vector.memset(nm4[:], 0)   # rows 0-3 overwritten by nm gathers

    # ---- load x: lower rows 1..128 <- x rows 0..127; upper slot r <- row r
    xh = d["xstack_d"].ap()[0:C, :, :]
    nc.scalar.dma_start(x2v[0:C, 1:129, 1:129], xh)
    nc.scalar.dma_start(x2v[C:2 * C, 0:128, 1:129], xh)

    # =====================================================================
    # Phase 1: mask tap channels s_t (exact, 4-term bf16 split), M=18
    # =====================================================================
    xsf = d["xstack_d"].ap().rearrange("c h w -> c (h w)")
    for kb in range(NCHUNK // 4):
        xst = ring.tile([2 * C, 4 * CHUNK], bf, tag="xst")
        nc.sync.dma_start(xst[:], xsf[:, kb * 4 * CHUNK:(kb + 1) * 4 * CHUNK])
        for j in range(4):
            k = 4 * kb + j
            r0 = 4 * k + 1
            xc = xst[:, j * CHUNK:(j + 1) * CHUNK]
            ps = psS.tile([128, CHUNK], f32, tag="s")
            po = ps[0:18, :]
            # s = mwhi.x_hi + mwlo.x_lo  (MM A)  +  mwlo.x_hi + mwhi.x_lo (MM B)
            nc.tensor.matmul(po, mwA[:], xc, start=True, stop=False,
                             tile_position=(0, 0))
            nc.tensor.matmul(po, mwB[:], xc, start=False, stop=True,
                             tile_position=(0, 0))
            nc.scalar.copy(sv[0:18, r0:r0 + 4, 1:129], po)

    # =====================================================================
    # Phase 2: mask pipeline -> nm4 rows
    # =====================================================================
    mapsv = maps.rearrange("p (t c) -> p t c", c=H)
    for t in range(18):
        dy, dx = (t % 9) // 3 - 1, (t % 9) % 3 - 1
        src = sv[t:t + 1, 1 + dy:129 + dy, 1 + dx:129 + dx]
        nc.gpsimd.dma_start(mapsv[:, t, :], src)

    for mi in range(2):
        mbase = 9 * mi
        u1 = ops.tile([H, 4 * H], f32, tag="u1")
        nc.vector.tensor_add(u1[:], mapsv[:, mbase:mbase + 4, :],
                             mapsv[:, mbase + 4:mbase + 8, :])
        u2 = ops.tile([H, 2 * H], f32, tag="u2")
        nc.vector.tensor_add(u2[:], u1[:, 0:2 * H], u1[:, 2 * H:4 * H])
        u3 = ops.tile([H, H], f32, tag="u3")
        nc.vector.tensor_add(u3[:], u2[:, 0:H], u2[:, H:2 * H])
        logit = ops.tile([H, H], f32, tag="logit")
        nc.vector.scalar_tensor_tensor(
            logit[:], u3[:], mb[:, mi:mi + 1], mapsv[:, mbase + 8, :],
            op0=ALU.add, op1=ALU.add)
        p = ops.tile([H, H], f32, tag="p")
        nc.scalar.activation(p[:], logit[:], AF.Sigmoid)
        m = ops.tile([H, H], f32, tag="m")
        nc.vector.scalar_tensor_tensor(m[:], logit[:], 0.0, p[:],
                                       op0=ALU.is_ge, op1=ALU.mult)
        mhi = ops.tile([H, H], bf, tag="mhi")
        nc.vector.tensor_copy(mhi[:], m[:])
        mlo = ops.tile([H, H], bf, tag="mlo")
        nc.vector.scalar_tensor_tensor(mlo[:], mhi[:], -1.0, m[:],
                                       op0=ALU.mult, op1=ALU.add)
        # vertical gaussian (g0-prescaled) via 2 col tiles x 3 split terms
        pnv = psS.tile([H, H], f32, tag="s")
        for cti in range(2):
            c0, c1 = (0, C) if cti == 0 else (C, 2 * C)
            tp = (0, 0) if cti == 0 else (0, 64)
            po = pnv[c0:c1, :]
            lhi = syhi[:, mi * H + c0:mi * H + c1]
            llo = sylo[:, mi * H + c0:mi * H + c1]
            nc.tensor.matmul(po, lhi, mhi[:], start=True, stop=False,
                             tile_position=tp)
            nc.tensor.matmul(po, lhi, mlo[:], start=False, stop=False,
                             tile_position=tp)
            nc.tensor.matmul(po, llo, mhi[:], start=False, stop=True,
                             tile_position=tp)
        nmv = ops.tile([H, P], f32, tag="nmv_sb")
        nc.vector.memset(nmv[:, 0:1], 0)
        nc.vector.memset(nmv[:, P - 1:P], 0)
        nc.scalar.copy(nmv[:, 1:129], pnv[:])
        t2 = ops.tile([H, H], f32, tag="t2")
        nc.vector.tensor_add(t2[:], nmv[:, 0:128], nmv[:, 2:130])
        nm = ops.tile([H, H], f32, tag="nm")
        nc.vector.scalar_tensor_tensor(nm[:], t2[:], grat[:, mi:mi + 1],
                                       nmv[:, 1:129], op0=ALU.mult,
                                       op1=ALU.add)
        nmhi = ops.tile([H, H], bf, tag="nmhi")
        nc.vector.tensor_copy(nmhi[:], nm[:])
        nmlo = ops.tile([H, H], bf, tag="nmlo")
        nc.vector.scalar_tensor_tensor(nmlo[:], nmhi[:], -1.0, nm[:],
                                       op0=ALU.mult, op1=ALU.add)
        nc.sync.dma_start(nm4v[2 * mi:2 * mi + 1, :, :], nmhi[:])
        nc.sync.dma_start(nm4v[2 * mi + 1:2 * mi + 2, :, :], nmlo[:])

    # =====================================================================
    # conv helpers
    # =====================================================================
    def conv_chunk(src2v, wp, ws, ps, par, k):
        tp = (0, 0) if par == 0 else (0, 64)
        po = ps[0:C, :] if par == 0 else ps[C:2 * C, :]
        r0 = 4 * k + 1
        for kx in range(3):
            dx = kx - 1
            rhs = src2v[:, r0 - 1:r0 + 3, 1 + dx:129 + dx]
            nc.tensor.matmul(po, wp[:, kx * C:(kx + 1) * C], rhs,
                             start=(kx == 0), stop=False, tile_position=tp)
            rhs1 = src2v[:, r0 + 1:r0 + 5, 1 + dx:129 + dx]
            nc.tensor.matmul(po, ws[:, kx * C:(kx + 1) * C], rhs1,
                             start=False, stop=(kx == 2), tile_position=tp)

    def bcast_chunk(mi, ps, par, k):
        tp = (0, 0) if par == 0 else (0, 64)
        po = ps[0:C, :] if par == 0 else ps[C:2 * C, :]
        rhs = nm4v[:, 4 * k:4 * k + 4, :]
        nc.tensor.matmul(po, sel[:, mi * C:(mi + 1) * C], rhs,
                         start=True, stop=True, tile_position=tp)

    # =====================================================================
    # Phase 3: conv1 + bn1 + relu + *nm1 -> o2 (bf16, padded)
    # =====================================================================
    for kp in range(NCHUNK // 2):
        pA = psA.tile([2 * C, CHUNK], f32, tag="cv")
        pB = psB.tile([2 * C, CHUNK], f32, tag="nm")
        for par in range(2):
            k = 2 * kp + par
            conv_chunk(x2v, w1p, w1s, pA, par, k)
            bcast_chunk(0, pB, par, k)
        rS = stg.tile([2 * C, CHUNK], f32, tag="ev")
        nc.scalar.activation(rS[:], pA[:], AF.Relu, bias=b1dup[:, 0:1])
        for par in range(2):
            k = 2 * kp + par
            r0 = 4 * k + 1
            h0, h1 = (0, C) if par == 0 else (C, 2 * C)
            nc.vector.tensor_mul(
                o2v[0:C, r0:r0 + 4, 1:129],
                rS[h0:h1, :].rearrange("p (r c) -> p r c", c=W),
                pB[h0:h1, :].rearrange("p (r c) -> p r c", c=W))
        if kp % 4 == 3:
            # upper slot r <- out1 row r (= lower slot r+1); rows of this
            # quarter are all evicted by chunk 8q+7 (this kp)
            q = kp // 4
            nc.sync.dma_start(o2v[C:2 * C, 32 * q:32 * q + 32, 1:129],
                              o2v[0:C, 32 * q + 1:32 * q + 33, 1:129])

    # load xres (reuses scr; Tile orders it after the maps DMAs read s)
    nc.scalar.dma_start(xres, d["xres_d"].ap().rearrange("c h w -> c (h w)"))

    # =====================================================================
    # Phase 4: conv2 + bn2 + *nm2 + residual + relu -> out
    # =====================================================================
    outf = d["out_d"].ap().rearrange("c h w -> c (h w)")
    for kp in range(NCHUNK // 2):
        pA = psA.tile([2 * C, CHUNK], f32, tag="cv")
        pB = psB.tile([2 * C, CHUNK], f32, tag="nm")
        for par in range(2):
            k = 2 * kp + par
            conv_chunk(o2v, w2p, w2s, pA, par, k)
            bcast_chunk(1, pB, par, k)
        tS = stg.tile([2 * C, CHUNK], f32, tag="ev")
        nc.scalar.activation(tS[:], pA[:], AF.Identity, bias=b2dup[:, 0:1])
        for par in range(2):
            k = 2 * kp + par
            h0, h1 = (0, C) if par == 0 else (C, 2 * C)
            uc = stg.tile([C, CHUNK], f32, tag="uc")
            nc.vector.tensor_mul(uc[:], tS[h0:h1, :], pB[h0:h1, :])
            vc = stg.tile([C, CHUNK], f32, tag="vc")
            nc.vector.tensor_add(vc[:], uc[:],
                                 xres[:, k * CHUNK:(k + 1) * CHUNK])
            oc = stg.tile([C, CHUNK], f32, tag="oc")
            nc.scalar.activation(oc[:], vc[:], AF.Relu)
            nc.gpsimd.dma_start(outf[:, k * CHUNK:(k + 1) * CHUNK], oc[:])

    ctx.close()


def _host_prep(inputs):
    """Fold BN, split dtypes, build constant matrices."""
    f32 = np.float32
    x = np.asarray(inputs["x"], f32)
    N = x.shape[0]

    def fold(w, gamma, beta, mean, var):
        scale = (np.asarray(gamma, f32)
                 / np.sqrt(np.asarray(var, f32) + f32(EPS_BN)))
        wf = np.asarray(w, f32) * scale[:, None, None, None]
        b = np.asarray(beta, f32) - np.asarray(mean, f32) * scale
        return wf, b

    w1f, b1 = fold(inputs["conv1_w"], inputs["bn1_gamma"], inputs["bn1_beta"],
                   inputs["bn1_mean"], inputs["bn1_var"])
    w2f, b2 = fold(inputs["conv2_w"], inputs["bn2_gamma"], inputs["bn2_beta"],
                   inputs["bn2_mean"], inputs["bn2_var"])

    def pack_conv(wf):
        wp = np.zeros((3, 2 * C, C), f32)
        wsg = np.zeros((3, 2 * C, C), f32)
        for kx in range(3):
            wp[kx, 0:C] = wf[:, :, 0, kx].T      # ky=0 (dy=-1), lower half
            wp[kx, C:2 * C] = wf[:, :, 1, kx].T  # ky=1 (dy=0) via shifted dup
            wsg[kx, 0:C] = wf[:, :, 2, kx].T     # ky=2 (dy=+1); upper half 0
        return wp.astype(BF16), wsg.astype(BF16)

    w1p, w1s = pack_conv(w1f)
    w2p, w2s = pack_conv(w2f)

    mw = np.zeros((C, 18), f32)
    for t in range(9):
        ky, kx = t // 3, t % 3
        mw[:, t] = np.asarray(inputs["mask1_w"], f32)[0, :, ky, kx]
        mw[:, 9 + t] = np.asarray(inputs["mask2_w"], f32)[0, :, ky, kx]
    mwhi, mwlo = _bf16_split(mw)
    mwA = np.concatenate([mwhi, mwlo], axis=0)
    mwB = np.concatenate([mwlo, mwhi], axis=0)

    syhi = np.zeros((2, H, H), BF16)
    sylo = np.zeros((2, H, H), BF16)
    grat = np.zeros((2, H), f32)
    for mi, sig in enumerate((inputs["sigma1"], inputs["sigma2"])):
        s = f32(np.asarray(sig, f32).reshape(-1)[0])
        dd = np.arange(3, dtype=f32) - f32(1.0)
        e = np.exp(-(dd * dd) / (2 * s * s)).astype(f32)
        g1d = (e / e.sum()).astype(f32)
        g0, g1 = g1d[1], g1d[0]
        sy = np.zeros((H, H), f32)
        for h in range(H):
            for dy in (-1, 0, 1):
                hh = h + dy
                if 0 <= hh < H:
                    sy[hh, h] = g0 * g1d[dy + 1]   # lhsT[h_in, h_out]
        hi, lo = _bf16_split(sy)
        syhi[mi], sylo[mi] = hi, lo
        grat[mi, :] = g1 / g0
    mb = np.zeros((2, H), f32)
    mb[0, :] = np.asarray(inputs["mask1_b"], f32).reshape(-1)[0]
    mb[1, :] = np.asarray(inputs["mask2_b"], f32).reshape(-1)[0]

    sel = np.zeros((2 * C, 2 * C), BF16)
    sel[0, 0:C] = 1
    sel[1, 0:C] = 1
    sel[2, C:2 * C] = 1
    sel[3, C:2 * C] = 1

    shared = dict(
        w1p=w1p, w1s=w1s, w2p=w2p, w2s=w2s,
        mwA=mwA, mwB=mwB, syhi=syhi, sylo=sylo, sel=sel,
        b1dup=np.concatenate([b1, b1]).astype(f32),
        b2dup=np.concatenate([b2, b2]).astype(f32),
        mb=mb, grat=grat,
    )
    per_core = []
    for i in range(N):
        xi = x[i]
        xhi = xi.astype(BF16)
        xlo = (xi - xhi.astype(f32)).astype(BF16)
        xstack = np.concatenate([xhi, xlo], axis=0)
        per_core.append(dict(shared, xstack=xstack, xres=xi))
    return per_core


def kernel(**inputs):
    global _COMPILED
    if _COMPILED is None:
        _COMPILED = _build()
    nc = _COMPILED
    per_core = _host_prep(inputs)
    res = bass_utils.run_bass_kernel_spmd(
        nc, per_core, core_ids=list(range(len(per_core))))
    out = np.stack([res.results[i]["out"] for i in range(len(per_core))])
    return out.astype(np.float32)


if __name__ == "__main__":
    sys.path.insert(0, os.path.dirname(os.path.abspath(__file__)))
    import jax
    import reference
    with jax.default_device(jax.devices("cpu")[0]):
        ins = {k: np.asarray(v) for k, v in reference.setup_inputs().items()}
        exp = np.asarray(reference.reference(**ins))
    act = kernel(**ins)
    err = np.abs(act - exp)
    denom = np.abs(exp).max()
    print(f"max abs err: {err.max():.3e}  (ref scale {denom:.3f})")
    print(f"Relative error: {err.max() / denom:.3e}")
